# revision 18
# baseline (speedup 1.0000x reference)
"""Bass/Trainium2 kernel for nn_LinearMultiheadAttention_75204877353238.

Math: the reference einsums share no indices between the activation and the
weight operands, so the whole module collapses to

    a_h     = sum(q_weights[h])                      (scalar per head)
    c_h     = D * sum(v_weights[h])                  (scalar per head)
    vsum[b,v] = sum_s v[b,s,v]
    g[b,h,s]  = sum_d softmax_s(a_h * q[b,s,d])[s,d]
    t[b,h,s]  = c_h * g[b,h,s]
    out[b,s,v] = max_h t[b,h,s] * vsum[b,v]
               = relu(vsum)[v]*max_h t[b,h,s] + (-relu(-vsum))[v]*min_h t[b,h,s]

k and k_weights are mathematically unused (the k-softmax is summed over its
normalization axis, which gives exactly 1).

Sharding: 8 cores; core c handles batch c//2 and head group c%2 (4 heads).
Host combines the two per-core partial head-extremes with np.maximum.

Host-side prep per core (tiny vs the 32MB tensors): q transposed to [D,S]
(layout only), v downcast to bf16, per-head scalars a/-a/c and per-column
max/min of q packed into a [128,16] consts tile. All O(S*D*H) math (exp,
normalization matvecs, output assembly) plus the full q read / out write
stay on device.

Per-core pipeline (engine balance, ~57us ACT exp is the floor):
  DMA  : qT blocked loads (exp chases them), v loads, out stores, t-row relayout
  ACT  : 8 full-tile exp passes with fused Z row-sum (accum_out), half the
         out-stage PSUM->SBUF copies (tail only)
  PE   : per-chunk t matvecs (bf16 eT stationary / FWL, rc moving), vsum
         ones-matvec, t-row transposes, rank-2 out-stage matmuls
  DVE  : negm/recip/rc smalls, running head max/min trees, other half of the
         out-stage copies
"""

import numpy as np

import concourse.bacc as bacc
import concourse.bass as bass
import concourse.mybir as mybir
import concourse.tile as tile
from concourse.bass_utils import run_bass_kernel_spmd
from concourse.masks import make_identity

B, S, D, H = 4, 8192, 256, 8
P = 128
NCORES = 8
HPC = H // 2            # heads per core
ND = D // P             # 2 d-tiles
NCHUNK = S // P         # 64 s-chunks of 128
NBLK = 4                # q-tile DMA blocks
SBLK = S // NBLK        # 2048
B0BLKS = ((0, 1024), (1024, 1024), (2048, 2048), (4096, 4096))
NB = 4                  # v/out s-chunks per DMA batch
NBATCH = NCHUNK // NB   # 16 DMA batches
F32 = mybir.dt.float32
BF16 = mybir.dt.bfloat16
AF = mybir.ActivationFunctionType
ALU = mybir.AluOpType
AX = mybir.AxisListType
ts = bass.ts

NPBF16 = mybir.dt.np(BF16)

TRACE = False
LAST_RESULTS = None


DEBUG_TAPS = False


def _build_nc(repeat=1):
    nc = bacc.Bacc("TRN2", target_bir_lowering=False, debug=False)

    qTd = nc.dram_tensor("qT", [D, S], F32, kind="ExternalInput")
    vd = nc.dram_tensor("v", [S, D], BF16, kind="ExternalInput")
    consd = nc.dram_tensor("cons", [P, 18], F32, kind="ExternalInput")
    outd = nc.dram_tensor("out", [S, D], BF16, kind="ExternalOutput")
    trowd = nc.dram_tensor("trow", [2 * S], BF16, kind="Internal")
    taps = None
    if DEBUG_TAPS:
        taps = {
            "dbg_t": nc.dram_tensor("dbg_t", [2, P, NCHUNK], F32,
                                    kind="ExternalOutput"),
            "dbg_ti": nc.dram_tensor("dbg_ti", [2, S], BF16,
                                     kind="ExternalOutput"),
            "dbg_tps": nc.dram_tensor("dbg_tps", [HPC, P, NCHUNK], F32,
                                      kind="ExternalOutput"),
            "dbg_q": nc.dram_tensor("dbg_q", [ND, P, S], F32,
                                    kind="ExternalOutput"),
            "dbg_e": nc.dram_tensor("dbg_e", [ND, P, S], BF16,
                                    kind="ExternalOutput"),
            "dbg_rc": nc.dram_tensor("dbg_rc", [ND, HPC, P], F32,
                                     kind="ExternalOutput"),
        }

    with tile.TileContext(nc) as tc:
        for _ in range(repeat):
            _body(nc, tc, qTd, vd, consd, outd, trowd, taps)

    nc.compile()
    return nc


def _body(nc, tc, qTd, vd, consd, outd, trowd, taps=None):
    qv = qTd.rearrange("(t p) s -> t p s", p=P)              # [2,128,8192]
    vd4 = vd.rearrange("(i n p) d -> i p n d", p=P, n=NB)    # [16,128,4,256]
    outd4 = outd.rearrange("(g q p) d -> g p q d", p=P, q=2)  # [32,128,2,256]
    trow2 = trowd.rearrange("(r s) -> r s", r=2)             # [2, 8192]
    trow_flat = trowd.rearrange("(r j f) -> (r j) f", f=P, r=2)  # [128,128]

    with (
        tc.tile_pool(name="consts", bufs=1) as consts,
        tc.tile_pool(name="qpool", bufs=1) as qpool,
        tc.tile_pool(name="etp", bufs=2) as etp,
        tc.tile_pool(name="vio", bufs=2) as vio,
        tc.tile_pool(name="oio", bufs=3) as oio,
        tc.tile_pool(name="small", bufs=2) as small,
        tc.tile_pool(name="tmerge", bufs=1) as tmerge,
        tc.tile_pool(name="pst", bufs=2, space="PSUM") as pst,
        tc.tile_pool(name="psv", bufs=1, space="PSUM") as psv,
        tc.tile_pool(name="psc", bufs=1, space="PSUM") as psc,
        tc.tile_pool(name="pso", bufs=3, space="PSUM") as pso,
    ):
        # ---- constants / ACT table warm-up ----
        cons = consts.tile([P, 18], F32, tag="cons")
        nc.sync.dma_start(cons, consd[:, :])
        ident = consts.tile([P, P], F32, tag="ident")
        make_identity(nc, ident)
        onesb = consts.tile([P, 1], BF16, tag="onesb")
        nc.vector.memset(onesb, 1.0)
        ones2 = consts.tile([1, 2], F32, tag="ones2")
        nc.vector.memset(ones2, 1.0)
        zdum = consts.tile([P, 1], F32, tag="zdum")
        nc.vector.memset(zdum, 0.0)
        warm = small.tile([P, 1], F32, tag="warm", bufs=1, name="warm")
        nc.scalar.activation(warm, zdum, AF.Exp)  # pull exp table load early

        # ---- qT blocked loads (tile-major; exp chases tile 0's blocks) ----
        qTt = [qpool.tile([P, S], F32, tag=f"qTt{t}", name=f"qTt{t}")
               for t in range(ND)]
        for off, ln in B0BLKS:
            nc.sync.dma_start(qTt[0][:, off:off + ln], qv[0][:, off:off + ln])
        for b_ in range(2):
            nc.sync.dma_start(qTt[1][:, ts(b_, S // 2)], qv[1][:, ts(b_, S // 2)])

        # ---- per (head, tile) exp bias: -max(a*cmax, a*cmin) ----
        # cons cols: 0-3 a_h, 4-7 -a_h, 8-11 c_h, 12-13 cmax(t), 14-15 cmin(t)
        negm = [[None] * ND for _ in range(HPC)]
        for h in range(HPC):
            for t in range(ND):
                mp = small.tile([P, 1], F32, tag="mp", bufs=4, name=f"mp{h}{t}")
                nc.vector.tensor_tensor(mp, cons[:, 4 + h:5 + h],
                                        cons[:, 12 + t:13 + t], op=ALU.mult)
                mn = small.tile([P, 1], F32, tag="mn", bufs=4, name=f"mn{h}{t}")
                nc.vector.tensor_tensor(mn, cons[:, 4 + h:5 + h],
                                        cons[:, 14 + t:15 + t], op=ALU.mult)
                nm = small.tile([P, 1], F32, tag=f"negm{h}{t}", bufs=1,
                                name=f"negm{h}{t}")
                nc.vector.tensor_tensor(nm, mp, mn, op=ALU.min)
                negm[h][t] = nm

        # ---- v loads + vsum ones-matvec (during the exp phase) ----
        vs_ps = psv.tile([1, 2 * D], F32, tag="vs", name="vs_ps")
        with tc.tile_wait_until(0.030):
            for i in range(NBATCH):
                vt = vio.tile([P, NB, D], BF16, tag="vload", name=f"v{i}")
                nc.sync.dma_start(vt, vd4[i])
                vf = vt.rearrange("p n d -> p (n d)")
                for half in range(2):
                    nc.tensor.matmul(vs_ps, onesb, vf[:, ts(half, 2 * D)],
                                     start=(i == 0 and half == 0),
                                     stop=(i == NBATCH - 1 and half == 1))
        vs_sb = small.tile([1, 2 * D], F32, tag="vs_sb", bufs=1, name="vs_sb")
        nc.vector.tensor_copy(vs_sb, vs_ps)
        vs2 = small.tile([1, D], F32, tag="vs2", bufs=1, name="vs2")
        nc.vector.tensor_tensor(vs2, vs_sb[:, 0:D], vs_sb[:, D:2 * D],
                                op=ALU.add)
        vb_ps = psc.tile([2, D], F32, tag="scratch", bufs=2, name="vb_ps")
        nc.tensor.matmul(vb_ps, ones2, vs2)        # both rows = vsum
        # row0 = relu(vsum), row1 = -relu(-vsum), via 0.5*x +- 0.5*|x| with a
        # per-partition +-1 selector (DVE can't address partition base 1).
        hh = small.tile([2, D], F32, tag="hh", bufs=1, name="hh")
        nc.vector.tensor_scalar_mul(hh, vb_ps, 0.5)
        hn = small.tile([2, D], F32, tag="hn", bufs=1, name="hn")
        nc.vector.tensor_scalar_mul(hn, vb_ps, -0.5)
        habs = small.tile([2, D], F32, tag="habs", bufs=1, name="habs")
        nc.vector.tensor_tensor(habs, hh, hn, op=ALU.max)
        VB = consts.tile([2, D], BF16, tag="VB")
        nc.vector.scalar_tensor_tensor(VB, in0=habs, scalar=cons[0:2, 16:17],
                                       in1=hh, op0=ALU.mult, op1=ALU.add)

        # ---- main loop: per d-tile, per head: exp (+Z), rc, t matvecs ----
        # Each (tile, head) matvec burst is its own single-shot PSUM group
        # (start=stop=True per column), spilled to SBUF; the d0+d1 add runs
        # on DVE. PSUM has_written accumulation across distant groups was
        # observed to drop contributions on HW (value-dependent), so it is
        # avoided entirely.
        tsb = [[None] * HPC for _ in range(ND)]
        tmx = tmerge.tile([P, NCHUNK], F32, tag="tmx", name="tmx")
        tmn = tmerge.tile([P, NCHUNK], F32, tag="tmn", name="tmn")
        for t in range(ND):
            for h in range(HPC):
                e = etp.tile([P, S], BF16, tag="eT", name=f"e{h}_{t}")
                if t == 0 and h == 0:
                    nb0 = len(B0BLKS)
                    z4 = small.tile([P, nb0], F32, tag="z4", bufs=1, name="z4")
                    for b_, (off, ln) in enumerate(B0BLKS):
                        nc.scalar.activation(
                            e[:, off:off + ln], qTt[0][:, off:off + ln],
                            AF.Exp, bias=negm[0][0], scale=cons[:, 0:1],
                            accum_out=z4[:, b_:b_ + 1])
                    z = small.tile([P, 1], F32, tag="z", bufs=2, name="z00")
                    nc.vector.tensor_reduce(z, z4, axis=AX.X, op=ALU.add)
                else:
                    z = small.tile([P, 1], F32, tag="z", bufs=2, name=f"z{h}{t}")
                    nc.scalar.activation(
                        e, qTt[t], AF.Exp, bias=negm[h][t],
                        scale=cons[:, h:h + 1], accum_out=z)
                if taps is not None and h == 1:
                    nc.sync.dma_start(taps["dbg_e"][t], e)
                r = small.tile([P, 1], F32, tag="r", bufs=2, name=f"r{h}{t}")
                nc.vector.reciprocal(r, z)
                rcf = small.tile([P, 1], F32, tag="rcf", bufs=2, name=f"rcf{h}{t}")
                nc.vector.tensor_tensor(rcf, r, cons[:, 8 + h:9 + h], op=ALU.mult)
                rc = small.tile([P, 1], BF16, tag="rc", bufs=2, name=f"rc{h}{t}")
                nc.vector.tensor_copy(rc, rcf)
                if taps is not None:
                    nc.sync.dma_start(
                        taps["dbg_rc"].rearrange("t h (p o) -> t h p o", o=1)[t, h], rcf)
                if t == ND - 1 and h == HPC - 1:
                    # last head: group-pipelined matvec + fold + relayout +
                    # out-stage so stores overlap the remaining matvecs
                    _last_head_tail(nc, e, rc, tsb, tmx, tmn, ident, VB,
                                    pst, psc, pso, oio, small, tmerge,
                                    trow_flat, trow2, outd4)
                    continue
                tpsc = pst.tile([P, NCHUNK], F32, tag="tps", bufs=2,
                                name=f"tps{h}_{t}")
                for j in range(NCHUNK):
                    nc.tensor.matmul(tpsc[:, j:j + 1], e[:, ts(j, P)], rc,
                                     start=True, stop=True)
                sp = small.tile([P, NCHUNK], F32, tag=f"tsb{t}{h}", bufs=1,
                                name=f"tsb{t}{h}")
                nc.vector.tensor_copy(sp, tpsc)
                tsb[t][h] = sp
                if t == ND - 1:
                    # fold this head into the running extremes
                    tcu = small.tile([P, NCHUNK], F32, tag="tcu", bufs=2,
                                     name=f"tcu{h}")
                    nc.vector.tensor_tensor(tcu, tsb[0][h], tsb[1][h],
                                            op=ALU.add)
                    if h == 0:
                        nc.vector.tensor_copy(tmx, tcu)
                        nc.vector.tensor_copy(tmn, tcu)
                    else:
                        nc.vector.tensor_tensor(tmx, tmx, tcu, op=ALU.max)
                        nc.vector.tensor_tensor(tmn, tmn, tcu, op=ALU.min)
                    if taps is not None:
                        nc.sync.dma_start(taps["dbg_tps"][h], tcu)

        if taps is not None:
            nc.sync.dma_start(taps["dbg_t"][0], tmx)
            nc.sync.dma_start(taps["dbg_t"][1], tmn)
            for t_ in range(ND):
                nc.sync.dma_start(taps["dbg_q"][t_], qTt[t_])


def _last_head_tail(nc, e, rc, tsb, tmx, tmn, ident, VB, pst, psc, pso,
                    oio, small, tmerge, trow_flat, trow2, outd4):
    """Last (tile, head): per 16-chunk group, run the matvecs, fold into the
    extremes, transpose to rows, relayout via DRAM, and stream the rank-2
    out-stage + stores — all pipelined against the remaining groups."""
    NG = 4
    GC = NCHUNK // NG                 # 16 chunks per group
    GS = GC * P                       # 2048 s per group
    h = HPC - 1
    sp = small.tile([P, NCHUNK], F32, tag="tsb_last", bufs=1, name="tsb_last")
    TI2 = tmerge.tile([2, S], BF16, tag="TI2", name="TI2")
    for g in range(NG):
        tpsg = pst.tile([P, GC], F32, tag="tps", bufs=2, name=f"tps3g{g}")
        for jj in range(GC):
            j = g * GC + jj
            nc.tensor.matmul(tpsg[:, jj:jj + 1], e[:, ts(j, P)], rc,
                             start=True, stop=True)
        gsl = slice(g * GC, (g + 1) * GC)
        nc.vector.tensor_copy(sp[:, gsl], tpsg)
        tcu = small.tile([P, GC], F32, tag="tcu3", bufs=2, name=f"tcu3_{g}")
        nc.vector.tensor_tensor(tcu, tsb[0][h][:, gsl], sp[:, gsl], op=ALU.add)
        nc.vector.tensor_tensor(tmx[:, gsl], tmx[:, gsl], tcu, op=ALU.max)
        nc.vector.tensor_tensor(tmn[:, gsl], tmn[:, gsl], tcu, op=ALU.min)
        # transpose the group's extremes to chunk-major rows, bf16
        for idx, stat in enumerate((tmx, tmn)):
            trp = psc.tile([GC, P], F32, tag="scratch", bufs=2,
                           name=f"trp{idx}_{g}")
            nc.tensor.transpose(trp, stat[:, gsl], ident)
            rsb = small.tile([GC, P], BF16, tag="rsb", bufs=4,
                             name=f"rsb{idx}_{g}")
            nc.vector.tensor_copy(rsb, trp)
            base = idx * NCHUNK + g * GC
            nc.sync.dma_start(trow_flat[base:base + GC], rsb)
        nc.sync.dma_start(TI2[:, g * GS:(g + 1) * GS],
                          trow2[:, g * GS:(g + 1) * GS])
        # out chunks of this group
        for gg in range(GC // 2):
            og = g * (GC // 2) + gg
            op = pso.tile([P, 2, D], F32, tag="ops", bufs=3, name=f"op{og}")
            for k in range(2):
                j = 2 * og + k
                nc.tensor.matmul(op[:, k, :], TI2[:, ts(j, P)], VB,
                                 start=True, stop=True)
            ob = oio.tile([P, 2, D], BF16, tag="osb", name=f"ob{og}")
            if og % 2 == 0:
                nc.scalar.copy(ob, op)
            else:
                nc.vector.tensor_copy(ob, op)
            nc.sync.dma_start(outd4[og], ob)


def per_core_inputs(data):
    """Shard full inputs into the per-core input map (test/bench helper).

    Single source of truth for the sharding used by kernel(); data is any
    mapping with q, v, q_weights, v_weights full arrays.
    """
    q = np.asarray(data["q"], dtype=np.float32)
    v = np.asarray(data["v"], dtype=np.float32)
    qw = np.asarray(data["q_weights"], dtype=np.float32)
    vw = np.asarray(data["v_weights"], dtype=np.float32)

    qT = [np.ascontiguousarray(q[b].T) for b in range(B)]
    vb = [np.ascontiguousarray(v[b]).astype(NPBF16) for b in range(B)]
    cmax = [q[b].max(axis=0) for b in range(B)]
    cmin = [q[b].min(axis=0) for b in range(B)]
    a_all = qw.sum(axis=(1, 2))                      # [8]
    c_all = float(D) * vw.sum(axis=(1, 2))           # [8]

    out = {"qT": [], "v": [], "cons": []}
    for c in range(NCORES):
        b, hg = c // 2, c % 2
        a = a_all[hg * HPC:(hg + 1) * HPC]
        cc = c_all[hg * HPC:(hg + 1) * HPC]
        cons = np.zeros((P, 18), dtype=np.float32)
        cons[0, 16] = 1.0
        cons[1, 16] = -1.0
        cons[:, 0:4] = a[None, :]
        cons[:, 4:8] = -a[None, :]
        cons[:, 8:12] = cc[None, :]
        cons[:, 12] = cmax[b][0:P]
        cons[:, 13] = cmax[b][P:2 * P]
        cons[:, 14] = cmin[b][0:P]
        cons[:, 15] = cmin[b][P:2 * P]
        out["qT"].append(qT[b])
        out["v"].append(vb[b])
        out["cons"].append(cons)
    return out


_NC_CACHE = None


def _get_nc():
    global _NC_CACHE
    if _NC_CACHE is None:
        _NC_CACHE = _build_nc()
    return _NC_CACHE


def kernel(q, k, v, q_weights, k_weights, v_weights):
    global LAST_RESULTS
    data = {"q": q, "v": v, "q_weights": q_weights, "v_weights": v_weights}
    pc = per_core_inputs(data)

    nc = _get_nc()
    in_maps = [
        {"qT": pc["qT"][c], "v": pc["v"][c], "cons": pc["cons"][c]}
        for c in range(NCORES)
    ]

    res = run_bass_kernel_spmd(nc, in_maps, core_ids=list(range(NCORES)),
                               trace=TRACE)
    LAST_RESULTS = res
    outs = [np.asarray(r["out"]).astype(np.float32) for r in res.results]
    full = np.stack([np.maximum(outs[2 * b], outs[2 * b + 1]) for b in range(B)])
    return full


# revision 19
# speedup vs baseline: 1.0808x; 1.0808x over previous
"""Bass/Trainium2 kernel for nn_LinearMultiheadAttention_75204877353238.

Math: the reference einsums share no indices between the activation and the
weight operands, so the whole module collapses to

    a_h     = sum(q_weights[h])                      (scalar per head)
    c_h     = D * sum(v_weights[h])                  (scalar per head)
    vsum[b,v] = sum_s v[b,s,v]
    g[b,h,s]  = sum_d softmax_s(a_h * q[b,s,d])[s,d]
    t[b,h,s]  = c_h * g[b,h,s]
    out[b,s,v] = max_h t[b,h,s] * vsum[b,v]
               = relu(vsum)[v]*max_h t[b,h,s] + (-relu(-vsum))[v]*min_h t[b,h,s]

k and k_weights are mathematically unused (the k-softmax is summed over its
normalization axis, which gives exactly 1).

Sharding: 8 cores; core c handles batch c//2 and head group c%2 (4 heads).
Host combines the two per-core partial head-extremes with np.maximum.

Host-side prep per core (tiny vs the 32MB tensors): q transposed to [D,S]
(layout only), v downcast to bf16, per-head scalars a/-a/c and per-column
max/min of q packed into a [128,16] consts tile. All O(S*D*H) math (exp,
normalization matvecs, output assembly) plus the full q read / out write
stay on device.

Per-core pipeline (engine balance, ~57us ACT exp is the floor):
  DMA  : qT blocked loads (exp chases them), v loads, out stores, t-row relayout
  ACT  : 8 full-tile exp passes with fused Z row-sum (accum_out), half the
         out-stage PSUM->SBUF copies (tail only)
  PE   : per-chunk t matvecs (bf16 eT stationary / FWL, rc moving), vsum
         ones-matvec, t-row transposes, rank-2 out-stage matmuls
  DVE  : negm/recip/rc smalls, running head max/min trees, other half of the
         out-stage copies
"""

import numpy as np

import concourse.bacc as bacc
import concourse.bass as bass
import concourse.mybir as mybir
import concourse.tile as tile
from concourse.bass_utils import run_bass_kernel_spmd
from concourse.masks import make_identity

B, S, D, H = 4, 8192, 256, 8
P = 128
NCORES = 8
HPC = H // 2            # heads per core
ND = D // P             # 2 d-tiles
NCHUNK = S // P         # 64 s-chunks of 128
NBLK = 4                # q-tile DMA blocks
SBLK = S // NBLK        # 2048
B0BLKS = ((0, 1024), (1024, 1024), (2048, 2048), (4096, 4096))
NB = 4                  # v/out s-chunks per DMA batch
NBATCH = NCHUNK // NB   # 16 DMA batches
F32 = mybir.dt.float32
BF16 = mybir.dt.bfloat16
AF = mybir.ActivationFunctionType
ALU = mybir.AluOpType
AX = mybir.AxisListType
ts = bass.ts

NPBF16 = mybir.dt.np(BF16)

TRACE = False
LAST_RESULTS = None


DEBUG_TAPS = False


def _build_nc(repeat=1):
    nc = bacc.Bacc("TRN2", target_bir_lowering=False, debug=False)

    qTd = nc.dram_tensor("qT", [D, S], F32, kind="ExternalInput")
    vd = nc.dram_tensor("v", [S, D], BF16, kind="ExternalInput")
    consd = nc.dram_tensor("cons", [P, 18], F32, kind="ExternalInput")
    outd = nc.dram_tensor("out", [S, D], BF16, kind="ExternalOutput")
    trowd = nc.dram_tensor("trow", [2 * S], BF16, kind="Internal")
    taps = None
    if DEBUG_TAPS:
        taps = {
            "dbg_t": nc.dram_tensor("dbg_t", [2, P, NCHUNK], F32,
                                    kind="ExternalOutput"),
            "dbg_ti": nc.dram_tensor("dbg_ti", [2, S], BF16,
                                     kind="ExternalOutput"),
            "dbg_tps": nc.dram_tensor("dbg_tps", [HPC, P, NCHUNK], F32,
                                      kind="ExternalOutput"),
            "dbg_q": nc.dram_tensor("dbg_q", [ND, P, S], F32,
                                    kind="ExternalOutput"),
            "dbg_e": nc.dram_tensor("dbg_e", [ND, P, S], BF16,
                                    kind="ExternalOutput"),
            "dbg_rc": nc.dram_tensor("dbg_rc", [ND, HPC, P], F32,
                                     kind="ExternalOutput"),
        }

    with tile.TileContext(nc) as tc:
        for _ in range(repeat):
            _body(nc, tc, qTd, vd, consd, outd, trowd, taps)

    nc.compile()
    return nc


def _body(nc, tc, qTd, vd, consd, outd, trowd, taps=None):
    qv = qTd.rearrange("(t p) s -> t p s", p=P)              # [2,128,8192]
    vd4 = vd.rearrange("(i n p) d -> i p n d", p=P, n=NB)    # [16,128,4,256]
    outd4 = outd.rearrange("(g q p) d -> g p q d", p=P, q=2)  # [32,128,2,256]
    trow2 = trowd.rearrange("(r s) -> r s", r=2)             # [2, 8192]
    trow_flat = trowd.rearrange("(r j f) -> (r j) f", f=P, r=2)  # [128,128]

    with (
        tc.tile_pool(name="consts", bufs=1) as consts,
        tc.tile_pool(name="qpool", bufs=1) as qpool,
        tc.tile_pool(name="etp", bufs=2) as etp,
        tc.tile_pool(name="vio", bufs=2) as vio,
        tc.tile_pool(name="oio", bufs=3) as oio,
        tc.tile_pool(name="small", bufs=2) as small,
        tc.tile_pool(name="tmerge", bufs=1) as tmerge,
        tc.tile_pool(name="pst", bufs=2, space="PSUM") as pst,
        tc.tile_pool(name="psv", bufs=1, space="PSUM") as psv,
        tc.tile_pool(name="psc", bufs=1, space="PSUM") as psc,
        tc.tile_pool(name="pso", bufs=3, space="PSUM") as pso,
    ):
        # ---- constants / ACT table warm-up ----
        cons = consts.tile([P, 18], F32, tag="cons")
        nc.sync.dma_start(cons, consd[:, :])
        ident = consts.tile([P, P], F32, tag="ident")
        make_identity(nc, ident)
        onesb = consts.tile([P, 1], BF16, tag="onesb")
        nc.vector.memset(onesb, 1.0)
        ones2 = consts.tile([1, 2], F32, tag="ones2")
        nc.vector.memset(ones2, 1.0)
        zdum = consts.tile([P, 1], F32, tag="zdum")
        nc.vector.memset(zdum, 0.0)
        warm = small.tile([P, 1], F32, tag="warm", bufs=1, name="warm")
        nc.scalar.activation(warm, zdum, AF.Exp)  # pull exp table load early

        # ---- qT blocked loads (tile-major; exp chases tile 0's blocks) ----
        qTt = [qpool.tile([P, S], F32, tag=f"qTt{t}", name=f"qTt{t}")
               for t in range(ND)]
        for off, ln in B0BLKS:
            nc.sync.dma_start(qTt[0][:, off:off + ln], qv[0][:, off:off + ln])
        for b_ in range(2):
            nc.sync.dma_start(qTt[1][:, ts(b_, S // 2)], qv[1][:, ts(b_, S // 2)])

        # ---- per (head, tile) exp bias: -max(a*cmax, a*cmin) ----
        # cons cols: 0-3 a_h, 4-7 -a_h, 8-11 c_h, 12-13 cmax(t), 14-15 cmin(t)
        negm = [[None] * ND for _ in range(HPC)]
        for h in range(HPC):
            for t in range(ND):
                mp = small.tile([P, 1], F32, tag="mp", bufs=4, name=f"mp{h}{t}")
                nc.vector.tensor_tensor(mp, cons[:, 4 + h:5 + h],
                                        cons[:, 12 + t:13 + t], op=ALU.mult)
                mn = small.tile([P, 1], F32, tag="mn", bufs=4, name=f"mn{h}{t}")
                nc.vector.tensor_tensor(mn, cons[:, 4 + h:5 + h],
                                        cons[:, 14 + t:15 + t], op=ALU.mult)
                nm = small.tile([P, 1], F32, tag=f"negm{h}{t}", bufs=1,
                                name=f"negm{h}{t}")
                nc.vector.tensor_tensor(nm, mp, mn, op=ALU.min)
                negm[h][t] = nm

        # ---- v loads + vsum ones-matvec (during the exp phase) ----
        vs_ps = psv.tile([1, 2 * D], F32, tag="vs", name="vs_ps")
        with tc.tile_wait_until(0.030):
            for i in range(NBATCH):
                vt = vio.tile([P, NB, D], BF16, tag="vload", name=f"v{i}")
                nc.sync.dma_start(vt, vd4[i])
                vf = vt.rearrange("p n d -> p (n d)")
                for half in range(2):
                    nc.tensor.matmul(vs_ps, onesb, vf[:, ts(half, 2 * D)],
                                     start=(i == 0 and half == 0),
                                     stop=(i == NBATCH - 1 and half == 1))
        vs_sb = small.tile([1, 2 * D], F32, tag="vs_sb", bufs=1, name="vs_sb")
        nc.vector.tensor_copy(vs_sb, vs_ps)
        vs2 = small.tile([1, D], F32, tag="vs2", bufs=1, name="vs2")
        nc.vector.tensor_tensor(vs2, vs_sb[:, 0:D], vs_sb[:, D:2 * D],
                                op=ALU.add)
        vb_ps = psc.tile([2, D], F32, tag="scratch", bufs=2, name="vb_ps")
        nc.tensor.matmul(vb_ps, ones2, vs2)        # both rows = vsum
        # row0 = relu(vsum), row1 = -relu(-vsum), via 0.5*x +- 0.5*|x| with a
        # per-partition +-1 selector (DVE can't address partition base 1).
        hh = small.tile([2, D], F32, tag="hh", bufs=1, name="hh")
        nc.vector.tensor_scalar_mul(hh, vb_ps, 0.5)
        hn = small.tile([2, D], F32, tag="hn", bufs=1, name="hn")
        nc.vector.tensor_scalar_mul(hn, vb_ps, -0.5)
        habs = small.tile([2, D], F32, tag="habs", bufs=1, name="habs")
        nc.vector.tensor_tensor(habs, hh, hn, op=ALU.max)
        VB = consts.tile([2, D], BF16, tag="VB")
        nc.vector.scalar_tensor_tensor(VB, in0=habs, scalar=cons[0:2, 16:17],
                                       in1=hh, op0=ALU.mult, op1=ALU.add)

        # ---- main loop: per d-tile, per head: exp (+Z), rc, t matvecs ----
        # Each (tile, head) matvec burst is its own single-shot PSUM group
        # (start=stop=True per column), spilled to SBUF; the d0+d1 add runs
        # on DVE. PSUM has_written accumulation across distant groups was
        # observed to drop contributions on HW (value-dependent), so it is
        # avoided entirely.
        tsb = [[None] * HPC for _ in range(ND)]
        tmx = tmerge.tile([P, NCHUNK], F32, tag="tmx", name="tmx")
        tmn = tmerge.tile([P, NCHUNK], F32, tag="tmn", name="tmn")
        for t in range(ND):
            for h in range(HPC):
                e = etp.tile([P, S], BF16, tag="eT", name=f"e{h}_{t}")
                if t == 0 and h == 0:
                    nb0 = len(B0BLKS)
                    z4 = small.tile([P, nb0], F32, tag="z4", bufs=1, name="z4")
                    for b_, (off, ln) in enumerate(B0BLKS):
                        nc.scalar.activation(
                            e[:, off:off + ln], qTt[0][:, off:off + ln],
                            AF.Exp, bias=negm[0][0], scale=cons[:, 0:1],
                            accum_out=z4[:, b_:b_ + 1])
                    z = small.tile([P, 1], F32, tag="z", bufs=2, name="z00")
                    nc.vector.tensor_reduce(z, z4, axis=AX.X, op=ALU.add)
                else:
                    z = small.tile([P, 1], F32, tag="z", bufs=2, name=f"z{h}{t}")
                    nc.scalar.activation(
                        e, qTt[t], AF.Exp, bias=negm[h][t],
                        scale=cons[:, h:h + 1], accum_out=z)
                if taps is not None and h == 1:
                    nc.sync.dma_start(taps["dbg_e"][t], e)
                r = small.tile([P, 1], F32, tag="r", bufs=2, name=f"r{h}{t}")
                nc.vector.reciprocal(r, z)
                rcf = small.tile([P, 1], F32, tag="rcf", bufs=2, name=f"rcf{h}{t}")
                nc.vector.tensor_tensor(rcf, r, cons[:, 8 + h:9 + h], op=ALU.mult)
                rc = small.tile([P, 1], BF16, tag="rc", bufs=2, name=f"rc{h}{t}")
                nc.vector.tensor_copy(rc, rcf)
                if taps is not None:
                    nc.sync.dma_start(
                        taps["dbg_rc"].rearrange("t h (p o) -> t h p o", o=1)[t, h], rcf)
                tpsc = pst.tile([P, NCHUNK], F32, tag="tps", bufs=2,
                                name=f"tps{h}_{t}")
                for j in range(NCHUNK):
                    nc.tensor.matmul(tpsc[:, j:j + 1], e[:, ts(j, P)], rc,
                                     start=True, stop=True)
                sp = small.tile([P, NCHUNK], F32, tag=f"tsb{t}{h}", bufs=1,
                                name=f"tsb{t}{h}")
                nc.vector.tensor_copy(sp, tpsc)
                tsb[t][h] = sp
                if t == ND - 1:
                    # fold this head into the running extremes
                    tcu = small.tile([P, NCHUNK], F32, tag="tcu", bufs=2,
                                     name=f"tcu{h}")
                    nc.vector.tensor_tensor(tcu, tsb[0][h], tsb[1][h],
                                            op=ALU.add)
                    if h == 0:
                        nc.vector.tensor_copy(tmx, tcu)
                        nc.vector.tensor_copy(tmn, tcu)
                    else:
                        nc.vector.tensor_tensor(tmx, tmx, tcu, op=ALU.max)
                        nc.vector.tensor_tensor(tmn, tmn, tcu, op=ALU.min)
                    if taps is not None:
                        nc.sync.dma_start(taps["dbg_tps"][h], tcu)

        # ---- t extremes -> row layout [2, S] via transpose + DRAM relayout --
        TI2 = tmerge.tile([2, S], BF16, tag="TI2", name="TI2")
        for idx, stat in enumerate((tmx, tmn)):
            trp = psc.tile([NCHUNK, P], F32, tag="scratch", bufs=2,
                           name=f"trp{idx}")
            nc.tensor.transpose(trp, stat, ident)  # [64,128] chunk-major rows
            rsb = small.tile([NCHUNK, P], BF16, tag="rsb", bufs=2,
                             name=f"rsb{idx}")
            nc.scalar.copy(rsb, trp)
            nc.sync.dma_start(trow_flat[idx * NCHUNK:(idx + 1) * NCHUNK], rsb)
        nc.sync.dma_start(TI2, trow2)
        if taps is not None:
            nc.sync.dma_start(taps["dbg_t"][0], tmx)
            nc.sync.dma_start(taps["dbg_t"][1], tmn)
            for t_ in range(ND):
                nc.sync.dma_start(taps["dbg_q"][t_], qTt[t_])

        # ---- out stage: out[s,v] = tmax[s]*vbpos[v] + tmin[s]*vbneg[v] ----
        for g in range(NCHUNK // 2):
            op = pso.tile([P, 2, D], F32, tag="ops", name=f"op{g}")
            for k in range(2):
                j = 2 * g + k
                nc.tensor.matmul(op[:, k, :], TI2[:, ts(j, P)], VB,
                                 start=True, stop=True)
            ob = oio.tile([P, 2, D], BF16, tag="osb", name=f"ob{g}")
            if g % 2 == 0:
                nc.scalar.copy(ob, op)
            else:
                nc.vector.tensor_copy(ob, op)
            nc.sync.dma_start(outd4[g], ob)


def per_core_inputs(data):
    """Shard full inputs into the per-core input map (test/bench helper).

    Single source of truth for the sharding used by kernel(); data is any
    mapping with q, v, q_weights, v_weights full arrays.
    """
    q = np.asarray(data["q"], dtype=np.float32)
    v = np.asarray(data["v"], dtype=np.float32)
    qw = np.asarray(data["q_weights"], dtype=np.float32)
    vw = np.asarray(data["v_weights"], dtype=np.float32)

    qT = [np.ascontiguousarray(q[b].T) for b in range(B)]
    vb = [np.ascontiguousarray(v[b]).astype(NPBF16) for b in range(B)]
    cmax = [q[b].max(axis=0) for b in range(B)]
    cmin = [q[b].min(axis=0) for b in range(B)]
    a_all = qw.sum(axis=(1, 2))                      # [8]
    c_all = float(D) * vw.sum(axis=(1, 2))           # [8]

    out = {"qT": [], "v": [], "cons": []}
    for c in range(NCORES):
        b, hg = c // 2, c % 2
        a = a_all[hg * HPC:(hg + 1) * HPC]
        cc = c_all[hg * HPC:(hg + 1) * HPC]
        cons = np.zeros((P, 18), dtype=np.float32)
        cons[0, 16] = 1.0
        cons[1, 16] = -1.0
        cons[:, 0:4] = a[None, :]
        cons[:, 4:8] = -a[None, :]
        cons[:, 8:12] = cc[None, :]
        cons[:, 12] = cmax[b][0:P]
        cons[:, 13] = cmax[b][P:2 * P]
        cons[:, 14] = cmin[b][0:P]
        cons[:, 15] = cmin[b][P:2 * P]
        out["qT"].append(qT[b])
        out["v"].append(vb[b])
        out["cons"].append(cons)
    return out


_NC_CACHE = None


def _get_nc():
    global _NC_CACHE
    if _NC_CACHE is None:
        _NC_CACHE = _build_nc()
    return _NC_CACHE


def kernel(q, k, v, q_weights, k_weights, v_weights):
    global LAST_RESULTS
    data = {"q": q, "v": v, "q_weights": q_weights, "v_weights": v_weights}
    pc = per_core_inputs(data)

    nc = _get_nc()
    in_maps = [
        {"qT": pc["qT"][c], "v": pc["v"][c], "cons": pc["cons"][c]}
        for c in range(NCORES)
    ]

    res = run_bass_kernel_spmd(nc, in_maps, core_ids=list(range(NCORES)),
                               trace=TRACE)
    LAST_RESULTS = res
    outs = [np.asarray(r["out"]).astype(np.float32) for r in res.results]
    full = np.stack([np.maximum(outs[2 * b], outs[2 * b + 1]) for b in range(B)])
    return full


# revision 20
# speedup vs baseline: 3.1437x; 2.9087x over previous
"""Bass/Trainium2 kernel for nn_LinearMultiheadAttention_75204877353238.

Math: the reference einsums share no indices between the activation and the
weight operands, so the whole module collapses to

    a_h     = sum(q_weights[h])                      (scalar per head)
    c_h     = D * sum(v_weights[h])                  (scalar per head)
    vsum[b,v] = sum_s v[b,s,v]
    g[b,h,s]  = sum_d softmax_s(a_h * q[b,s,d])[s,d]
    t[b,h,s]  = c_h * g[b,h,s]
    out[b,s,v] = max_h t[b,h,s] * vsum[b,v]
               = relu(vsum)[v]*max_h t[b,h,s] + (-relu(-vsum))[v]*min_h t[b,h,s]

k and k_weights are mathematically unused (the k-softmax is summed over its
normalization axis, which gives exactly 1).

Sharding: 8 cores; core c handles batch c//2 and head group c%2 (4 heads).
Host combines the two per-core partial head-extremes with np.maximum.

Host-side prep per core (tiny vs the 32MB tensors): q transposed to [D,S]
(layout only), v downcast to bf16, per-head scalars a/-a/c and per-column
max/min of q packed into a [128,16] consts tile. All O(S*D*H) math (exp,
normalization matvecs, output assembly) plus the full q read / out write
stay on device.

Per-core pipeline (engine balance, ~57us ACT exp is the floor):
  DMA  : qT blocked loads (exp chases them), v loads, out stores, t-row relayout
  ACT  : 8 full-tile exp passes with fused Z row-sum (accum_out), half the
         out-stage PSUM->SBUF copies (tail only)
  PE   : per-chunk t matvecs (bf16 eT stationary / FWL, rc moving), vsum
         ones-matvec, t-row transposes, rank-2 out-stage matmuls
  DVE  : negm/recip/rc smalls, running head max/min trees, other half of the
         out-stage copies
"""

import numpy as np

import concourse.bacc as bacc
import concourse.bass as bass
import concourse.mybir as mybir
import concourse.tile as tile
from concourse.bass_utils import run_bass_kernel_spmd
from concourse.masks import make_identity

B, S, D, H = 4, 8192, 256, 8
P = 128
NCORES = 8
HPC = H // 2            # heads per core
ND = D // P             # 2 d-tiles
NCHUNK = S // P         # 64 s-chunks of 128
NBLK = 4                # q-tile DMA blocks
SBLK = S // NBLK        # 2048
B0BLKS = ((0, 1024), (1024, 1024), (2048, 2048), (4096, 4096))
NB = 4                  # v/out s-chunks per DMA batch
NBATCH = NCHUNK // NB   # 16 DMA batches
F32 = mybir.dt.float32
BF16 = mybir.dt.bfloat16
AF = mybir.ActivationFunctionType
ALU = mybir.AluOpType
AX = mybir.AxisListType
ts = bass.ts

NPBF16 = mybir.dt.np(BF16)

TRACE = False
LAST_RESULTS = None
VARIANT = "full"  # full | exponly | noout


DEBUG_TAPS = False


def _build_nc(repeat=1):
    nc = bacc.Bacc("TRN2", target_bir_lowering=False, debug=False)

    qTd = nc.dram_tensor("qT", [D, S], F32, kind="ExternalInput")
    vd = nc.dram_tensor("v", [S, D], BF16, kind="ExternalInput")
    consd = nc.dram_tensor("cons", [P, 18], F32, kind="ExternalInput")
    outd = nc.dram_tensor("out", [S, D], BF16, kind="ExternalOutput")
    trowd = nc.dram_tensor("trow", [2 * S], BF16, kind="Internal")
    taps = None
    if DEBUG_TAPS:
        taps = {
            "dbg_t": nc.dram_tensor("dbg_t", [2, P, NCHUNK], F32,
                                    kind="ExternalOutput"),
            "dbg_ti": nc.dram_tensor("dbg_ti", [2, S], BF16,
                                     kind="ExternalOutput"),
            "dbg_tps": nc.dram_tensor("dbg_tps", [HPC, P, NCHUNK], F32,
                                      kind="ExternalOutput"),
            "dbg_q": nc.dram_tensor("dbg_q", [ND, P, S], F32,
                                    kind="ExternalOutput"),
            "dbg_e": nc.dram_tensor("dbg_e", [ND, P, S], BF16,
                                    kind="ExternalOutput"),
            "dbg_rc": nc.dram_tensor("dbg_rc", [ND, HPC, P], F32,
                                     kind="ExternalOutput"),
        }

    with tile.TileContext(nc) as tc:
        for _ in range(repeat):
            _body(nc, tc, qTd, vd, consd, outd, trowd, taps)

    nc.compile()
    return nc


def _body(nc, tc, qTd, vd, consd, outd, trowd, taps=None):
    qv = qTd.rearrange("(t p) s -> t p s", p=P)              # [2,128,8192]
    vd4 = vd.rearrange("(i n p) d -> i p n d", p=P, n=NB)    # [16,128,4,256]
    outd4 = outd.rearrange("(g q p) d -> g p q d", p=P, q=2)  # [32,128,2,256]
    trow2 = trowd.rearrange("(r s) -> r s", r=2)             # [2, 8192]
    trow_flat = trowd.rearrange("(r j f) -> (r j) f", f=P, r=2)  # [128,128]

    with (
        tc.tile_pool(name="consts", bufs=1) as consts,
        tc.tile_pool(name="qpool", bufs=1) as qpool,
        tc.tile_pool(name="etp", bufs=2) as etp,
        tc.tile_pool(name="vio", bufs=2) as vio,
        tc.tile_pool(name="oio", bufs=3) as oio,
        tc.tile_pool(name="small", bufs=2) as small,
        tc.tile_pool(name="tmerge", bufs=1) as tmerge,
        tc.tile_pool(name="pst", bufs=2, space="PSUM") as pst,
        tc.tile_pool(name="psv", bufs=1, space="PSUM") as psv,
        tc.tile_pool(name="psc", bufs=1, space="PSUM") as psc,
        tc.tile_pool(name="pso", bufs=3, space="PSUM") as pso,
    ):
        # ---- constants / ACT table warm-up ----
        cons = consts.tile([P, 18], F32, tag="cons")
        nc.sync.dma_start(cons, consd[:, :])
        ident = consts.tile([P, P], F32, tag="ident")
        make_identity(nc, ident)
        onesb = consts.tile([P, 1], BF16, tag="onesb")
        nc.vector.memset(onesb, 1.0)
        ones2 = consts.tile([1, 2], F32, tag="ones2")
        nc.vector.memset(ones2, 1.0)
        zdum = consts.tile([P, 1], F32, tag="zdum")
        nc.vector.memset(zdum, 0.0)
        warm = small.tile([P, 1], F32, tag="warm", bufs=1, name="warm")
        nc.scalar.activation(warm, zdum, AF.Exp)  # pull exp table load early

        # ---- qT blocked loads (tile-major; exp chases tile 0's blocks) ----
        qTt = [qpool.tile([P, S], F32, tag=f"qTt{t}", name=f"qTt{t}")
               for t in range(ND)]
        for off, ln in B0BLKS:
            nc.sync.dma_start(qTt[0][:, off:off + ln], qv[0][:, off:off + ln])
        for b_ in range(2):
            nc.sync.dma_start(qTt[1][:, ts(b_, S // 2)], qv[1][:, ts(b_, S // 2)])

        # ---- per (head, tile) exp bias: -max(a*cmax, a*cmin) ----
        # cons cols: 0-3 a_h, 4-7 -a_h, 8-11 c_h, 12-13 cmax(t), 14-15 cmin(t)
        negm = [[None] * ND for _ in range(HPC)]
        for h in range(HPC):
            for t in range(ND):
                mp = small.tile([P, 1], F32, tag="mp", bufs=4, name=f"mp{h}{t}")
                nc.vector.tensor_tensor(mp, cons[:, 4 + h:5 + h],
                                        cons[:, 12 + t:13 + t], op=ALU.mult)
                mn = small.tile([P, 1], F32, tag="mn", bufs=4, name=f"mn{h}{t}")
                nc.vector.tensor_tensor(mn, cons[:, 4 + h:5 + h],
                                        cons[:, 14 + t:15 + t], op=ALU.mult)
                nm = small.tile([P, 1], F32, tag=f"negm{h}{t}", bufs=1,
                                name=f"negm{h}{t}")
                nc.vector.tensor_tensor(nm, mp, mn, op=ALU.min)
                negm[h][t] = nm

        # ---- v loads + vsum ones-matvec (during the exp phase) ----
        vs_ps = psv.tile([1, 2 * D], F32, tag="vs", name="vs_ps")
        with tc.tile_wait_until(0.030):
            for i in range(NBATCH):
                vt = vio.tile([P, NB, D], BF16, tag="vload", name=f"v{i}")
                nc.sync.dma_start(vt, vd4[i])
                vf = vt.rearrange("p n d -> p (n d)")
                for half in range(2):
                    nc.tensor.matmul(vs_ps, onesb, vf[:, ts(half, 2 * D)],
                                     start=(i == 0 and half == 0),
                                     stop=(i == NBATCH - 1 and half == 1))
        vs_sb = small.tile([1, 2 * D], F32, tag="vs_sb", bufs=1, name="vs_sb")
        nc.vector.tensor_copy(vs_sb, vs_ps)
        vs2 = small.tile([1, D], F32, tag="vs2", bufs=1, name="vs2")
        nc.vector.tensor_tensor(vs2, vs_sb[:, 0:D], vs_sb[:, D:2 * D],
                                op=ALU.add)
        vb_ps = psc.tile([2, D], F32, tag="scratch", bufs=2, name="vb_ps")
        nc.tensor.matmul(vb_ps, ones2, vs2)        # both rows = vsum
        # row0 = relu(vsum), row1 = -relu(-vsum), via 0.5*x +- 0.5*|x| with a
        # per-partition +-1 selector (DVE can't address partition base 1).
        hh = small.tile([2, D], F32, tag="hh", bufs=1, name="hh")
        nc.vector.tensor_scalar_mul(hh, vb_ps, 0.5)
        hn = small.tile([2, D], F32, tag="hn", bufs=1, name="hn")
        nc.vector.tensor_scalar_mul(hn, vb_ps, -0.5)
        habs = small.tile([2, D], F32, tag="habs", bufs=1, name="habs")
        nc.vector.tensor_tensor(habs, hh, hn, op=ALU.max)
        VB = consts.tile([2, D], BF16, tag="VB")
        nc.vector.scalar_tensor_tensor(VB, in0=habs, scalar=cons[0:2, 16:17],
                                       in1=hh, op0=ALU.mult, op1=ALU.add)

        # ---- main loop: per d-tile, per head: exp (+Z), rc, t matvecs ----
        # Each (tile, head) matvec burst is its own single-shot PSUM group
        # (start=stop=True per column), spilled to SBUF; the d0+d1 add runs
        # on DVE. PSUM has_written accumulation across distant groups was
        # observed to drop contributions on HW (value-dependent), so it is
        # avoided entirely.
        tsb = [[None] * HPC for _ in range(ND)]
        tmx = tmerge.tile([P, NCHUNK], F32, tag="tmx", name="tmx")
        tmn = tmerge.tile([P, NCHUNK], F32, tag="tmn", name="tmn")
        for t in range(ND):
            for h in range(HPC):
                e = etp.tile([P, S], BF16, tag="eT", name=f"e{h}_{t}")
                if t == 0 and h == 0:
                    nb0 = len(B0BLKS)
                    z4 = small.tile([P, nb0], F32, tag="z4", bufs=1, name="z4")
                    for b_, (off, ln) in enumerate(B0BLKS):
                        nc.scalar.activation(
                            e[:, off:off + ln], qTt[0][:, off:off + ln],
                            AF.Exp, bias=negm[0][0], scale=cons[:, 0:1],
                            accum_out=z4[:, b_:b_ + 1])
                    z = small.tile([P, 1], F32, tag="z", bufs=2, name="z00")
                    nc.vector.tensor_reduce(z, z4, axis=AX.X, op=ALU.add)
                else:
                    z = small.tile([P, 1], F32, tag="z", bufs=2, name=f"z{h}{t}")
                    nc.scalar.activation(
                        e, qTt[t], AF.Exp, bias=negm[h][t],
                        scale=cons[:, h:h + 1], accum_out=z)
                if taps is not None and h == 1:
                    nc.sync.dma_start(taps["dbg_e"][t], e)
                r = small.tile([P, 1], F32, tag="r", bufs=2, name=f"r{h}{t}")
                nc.vector.reciprocal(r, z)
                rcf = small.tile([P, 1], F32, tag="rcf", bufs=2, name=f"rcf{h}{t}")
                nc.vector.tensor_tensor(rcf, r, cons[:, 8 + h:9 + h], op=ALU.mult)
                rc = small.tile([P, 1], BF16, tag="rc", bufs=2, name=f"rc{h}{t}")
                nc.vector.tensor_copy(rc, rcf)
                if VARIANT == "exponly":
                    continue
                if taps is not None:
                    nc.sync.dma_start(
                        taps["dbg_rc"].rearrange("t h (p o) -> t h p o", o=1)[t, h], rcf)
                tpsc = pst.tile([P, NCHUNK], F32, tag="tps", bufs=2,
                                name=f"tps{h}_{t}")
                for j in range(NCHUNK):
                    nc.tensor.matmul(tpsc[:, j:j + 1], e[:, ts(j, P)], rc,
                                     start=True, stop=True)
                sp = small.tile([P, NCHUNK], F32, tag=f"tsb{t}{h}", bufs=1,
                                name=f"tsb{t}{h}")
                nc.vector.tensor_copy(sp, tpsc)
                tsb[t][h] = sp
                if t == ND - 1:
                    # fold this head into the running extremes
                    tcu = small.tile([P, NCHUNK], F32, tag="tcu", bufs=2,
                                     name=f"tcu{h}")
                    nc.vector.tensor_tensor(tcu, tsb[0][h], tsb[1][h],
                                            op=ALU.add)
                    if h == 0:
                        nc.vector.tensor_copy(tmx, tcu)
                        nc.vector.tensor_copy(tmn, tcu)
                    else:
                        nc.vector.tensor_tensor(tmx, tmx, tcu, op=ALU.max)
                        nc.vector.tensor_tensor(tmn, tmn, tcu, op=ALU.min)
                    if taps is not None:
                        nc.sync.dma_start(taps["dbg_tps"][h], tcu)

        if VARIANT in ("exponly", "noout"):
            return
        # ---- t extremes -> row layout [2, S] via transpose + DRAM relayout --
        TI2 = tmerge.tile([2, S], BF16, tag="TI2", name="TI2")
        for idx, stat in enumerate((tmx, tmn)):
            trp = psc.tile([NCHUNK, P], F32, tag="scratch", bufs=2,
                           name=f"trp{idx}")
            nc.tensor.transpose(trp, stat, ident)  # [64,128] chunk-major rows
            rsb = small.tile([NCHUNK, P], BF16, tag="rsb", bufs=2,
                             name=f"rsb{idx}")
            nc.scalar.copy(rsb, trp)
            nc.sync.dma_start(trow_flat[idx * NCHUNK:(idx + 1) * NCHUNK], rsb)
        nc.sync.dma_start(TI2, trow2)
        if taps is not None:
            nc.sync.dma_start(taps["dbg_t"][0], tmx)
            nc.sync.dma_start(taps["dbg_t"][1], tmn)
            for t_ in range(ND):
                nc.sync.dma_start(taps["dbg_q"][t_], qTt[t_])

        # ---- out stage: out[s,v] = tmax[s]*vbpos[v] + tmin[s]*vbneg[v] ----
        for g in range(NCHUNK // 2):
            op = pso.tile([P, 2, D], F32, tag="ops", name=f"op{g}")
            for k in range(2):
                j = 2 * g + k
                nc.tensor.matmul(op[:, k, :], TI2[:, ts(j, P)], VB,
                                 start=True, stop=True)
            ob = oio.tile([P, 2, D], BF16, tag="osb", name=f"ob{g}")
            if g % 2 == 0:
                nc.scalar.copy(ob, op)
            else:
                nc.vector.tensor_copy(ob, op)
            nc.sync.dma_start(outd4[g], ob)


def per_core_inputs(data):
    """Shard full inputs into the per-core input map (test/bench helper).

    Single source of truth for the sharding used by kernel(); data is any
    mapping with q, v, q_weights, v_weights full arrays.
    """
    q = np.asarray(data["q"], dtype=np.float32)
    v = np.asarray(data["v"], dtype=np.float32)
    qw = np.asarray(data["q_weights"], dtype=np.float32)
    vw = np.asarray(data["v_weights"], dtype=np.float32)

    qT = [np.ascontiguousarray(q[b].T) for b in range(B)]
    vb = [np.ascontiguousarray(v[b]).astype(NPBF16) for b in range(B)]
    cmax = [q[b].max(axis=0) for b in range(B)]
    cmin = [q[b].min(axis=0) for b in range(B)]
    a_all = qw.sum(axis=(1, 2))                      # [8]
    c_all = float(D) * vw.sum(axis=(1, 2))           # [8]

    out = {"qT": [], "v": [], "cons": []}
    for c in range(NCORES):
        b, hg = c // 2, c % 2
        a = a_all[hg * HPC:(hg + 1) * HPC]
        cc = c_all[hg * HPC:(hg + 1) * HPC]
        cons = np.zeros((P, 18), dtype=np.float32)
        cons[0, 16] = 1.0
        cons[1, 16] = -1.0
        cons[:, 0:4] = a[None, :]
        cons[:, 4:8] = -a[None, :]
        cons[:, 8:12] = cc[None, :]
        cons[:, 12] = cmax[b][0:P]
        cons[:, 13] = cmax[b][P:2 * P]
        cons[:, 14] = cmin[b][0:P]
        cons[:, 15] = cmin[b][P:2 * P]
        out["qT"].append(qT[b])
        out["v"].append(vb[b])
        out["cons"].append(cons)
    return out


_NC_CACHE = None


def _get_nc():
    global _NC_CACHE
    if _NC_CACHE is None:
        _NC_CACHE = _build_nc()
    return _NC_CACHE


def kernel(q, k, v, q_weights, k_weights, v_weights):
    global LAST_RESULTS
    data = {"q": q, "v": v, "q_weights": q_weights, "v_weights": v_weights}
    pc = per_core_inputs(data)

    nc = _get_nc()
    in_maps = [
        {"qT": pc["qT"][c], "v": pc["v"][c], "cons": pc["cons"][c]}
        for c in range(NCORES)
    ]

    res = run_bass_kernel_spmd(nc, in_maps, core_ids=list(range(NCORES)),
                               trace=TRACE)
    LAST_RESULTS = res
    outs = [np.asarray(r["out"]).astype(np.float32) for r in res.results]
    full = np.stack([np.maximum(outs[2 * b], outs[2 * b + 1]) for b in range(B)])
    return full
